# revision 63
# baseline (speedup 1.0000x reference)
"""Trainium2 Bass kernel for a 2-layer Longformer-style encoder.

Model: B=2, S=2048, F=438, H=768, NH=12, HD=64, one-sided window w=32, L=2.

Sharding: 8 cores, data-parallel over (batch, sequence-quarter). Each core
computes 512 output tokens from a 640-token local window (64-token halo on
each side covers the 2-layer receptive field), so no collectives are needed.

Device algorithm per core (uniform SPMD, 640 local tokens):
  - x0 = srcT_pad.T @ W_embT + (pos_emb + b_emb)           [token-major f32]
  - per layer:
      xT   = transpose(x)  bf16                             [feature-major]
      qT   = W_qT'.T @ xT (+bq'), scaled by HD^-0.5 on host [feature-major]
      kTp  = W_kT.T @ xT (+bk), written at free-offset 32 into a
             768-wide padded buffer                         [feature-major]
      V_sh = shifted-window GEMM: tile i holds local tokens
             [128i-32, 128i+96)                             [token-major]
      per (query tile t, head pair hp, head half hs):
        2 score matmuls -> stp [128, 256] PSUM = [A | B'] where A covers
             keys [128t-32,128t+96) and B' a full 128 keys
             [128t+96,128t+224) so every stp element is a real score
             (one stationary row-class per PSUM bank - required: mixing
             row tile-positions in one bank hard-faults the device)
        p = exp(stp) in ONE [128,256] ACT (bf16 out), p *= mask01 (DVE)
        den += two K=128 ones-matmuls; ctx += V_sh[t|t+1].T @ p
        per hp: clamp den, rb = 1/den on DVE (not ACT: the Reciprocal
             LUT would thrash the Exp table set), ctxT = ctx * rb
      fc: F = ctxT.T @ W_fcT + residual; LN1 -> x1; transpose -> x1T bf16
      H1T = relu(W_1T.T @ x1T + b1)                         [feature-major]
      F2 = H1T.T @ W_2T (+b2) + x1; LN2 -> x2
  - out = x2[64:576]

Engine split: PE streams GEMMs; Scalar does exp, q-drains, half the relu
drains (Exp/Relu/Copy share one LUT set -> ~4 table loads total, vs 130
when ACT-Reciprocal was in the loop); Vector does k/v drains, masks, LN,
reciprocal. Measured on HW: 539.7us (predecessor) -> 312.8us.
"""

import numpy as np
import ml_dtypes

B, S, F_DIM, H, NH, HD, W_ONE, L = 2, 2048, 438, 768, 12, 64, 32, 2
NCORES = 8
CHUNK = 512          # output tokens per core
HALO = 64            # per side
T_LOC = CHUNK + 2 * HALO   # 640 local tokens
NT = T_LOC // 128          # 5 query tiles
KPAD = 768                 # padded key width (needs 128t+256 <= KPAD at t=4)
FK = 512                   # padded embedding contraction (438 -> 512)

bf16 = ml_dtypes.bfloat16

# Route mask-mult/memsets to the Pool (GpSimd) engine; flip off to bisect
# HW-only failures (CoreSim accepts both).
USE_GPSIMD = False
EPS_MATMUL = False     # den += 1e-6 via K=1 matmul (else: vector clamp)
SCALAR_TR_DRAIN = False  # 3-dim ACT Copy for transpose drains (else vector)
ATTN_DEPTH = 99
TT_DIVIDE = False       # single TT divide instead of recip+mult        # bisect: 1=scores 2=+exp 3=+mask 4=+den/ctx 5=+recip 6=all
STP_SHARED = False      # scores share the transpose PSUM pool (else own pool)
B_PRIME = False         # emit the B' (extended second-span) score matmuls


def _np(x):
    return np.asarray(x)


def host_prep(inputs):
    """Split full inputs into shared weight arrays + per-core arrays."""
    src_seq = _np(inputs["src_seq"]).astype(np.float32)
    src_pos = _np(inputs["src_pos"]).astype(np.int32)
    pos_table = _np(inputs["pos_table"]).astype(np.float32)

    shared = {}
    qscale = float(HD) ** -0.5

    W_emb = _np(inputs["W_emb"]).astype(np.float32)        # [H, F]
    WembT = np.zeros((FK, H), np.float32)
    WembT[:F_DIM] = W_emb.T
    shared["wembT"] = WembT.astype(bf16)

    for l in range(L):
        Wq = _np(inputs["Wq"])[l].astype(np.float32)
        Wk = _np(inputs["Wk"])[l].astype(np.float32)
        Wv = _np(inputs["Wv"])[l].astype(np.float32)
        Wfc = _np(inputs["Wfc"])[l].astype(np.float32)
        W1 = _np(inputs["W1"])[l].astype(np.float32)
        W2 = _np(inputs["W2"])[l].astype(np.float32)
        shared[f"wqT{l}"] = (Wq.T * qscale).astype(bf16)   # [H_in, H_out]
        shared[f"wkT{l}"] = Wk.T.astype(bf16)
        shared[f"wvT{l}"] = Wv.T.astype(bf16)
        shared[f"wfcT{l}"] = Wfc.T.astype(bf16)
        shared[f"w1T{l}"] = W1.T.astype(bf16)
        shared[f"w2T{l}"] = W2.T.astype(bf16)
        shared[f"bq{l}"] = (_np(inputs["bq"])[l].astype(np.float32) * qscale)
        shared[f"bk{l}"] = _np(inputs["bk"])[l].astype(np.float32)
        shared[f"bv{l}"] = _np(inputs["bv"])[l].astype(np.float32)
        shared[f"bfc{l}"] = _np(inputs["bfc"])[l].astype(np.float32)
        shared[f"b1{l}"] = _np(inputs["b1"])[l].astype(np.float32)
        shared[f"b2{l}"] = _np(inputs["b2"])[l].astype(np.float32)
        shared[f"ln1g{l}"] = _np(inputs["ln1_g"])[l].astype(np.float32)
        shared[f"ln1b{l}"] = _np(inputs["ln1_b"])[l].astype(np.float32)
        shared[f"ln2g{l}"] = _np(inputs["ln2_g"])[l].astype(np.float32)
        shared[f"ln2b{l}"] = _np(inputs["ln2_b"])[l].astype(np.float32)

    b_emb = _np(inputs["b_emb"]).astype(np.float32)

    per_core = []
    for c in range(NCORES):
        b, q = divmod(c, NCORES // B)
        gstart = q * CHUNK - HALO
        lo, hi = max(gstart, 0), min(gstart + T_LOC, S)

        src_halo = np.zeros((T_LOC, F_DIM), np.float32)
        src_halo[lo - gstart: hi - gstart] = src_seq[b, lo:hi]
        srcT = np.zeros((FK, T_LOC), np.float32)
        srcT[:F_DIM] = src_halo.T

        pos_emb = np.zeros((T_LOC, H), np.float32)
        pos_emb[lo - gstart: hi - gstart] = pos_table[src_pos[b, lo:hi]]
        pos_emb += b_emb[None, :]

        # multiplicative 0/1 mask, n-major, [n, region(A|B'), q] per tile,
        # duplicated for the two heads of a feature tile: [128, NT, 512]
        # A : key_local = 128t - 32 + n  (n in [0,128))
        # B': key_local = 128t + 96 + n  (n in [0,128))
        mask01 = np.zeros((128, NT, 4, 128), np.float32)
        n = np.arange(128)[:, None]
        qq = np.arange(128)[None, :]
        for t in range(NT):
            for r, kbase in ((0, 128 * t - 32), (1, 128 * t + 96)):
                kl = kbase + n
                kg = gstart + kl
                band = np.abs(kl - (128 * t + qq)) <= W_ONE
                valid = band & (kl >= 0) & (kl < T_LOC) & (kg >= 0) & (kg < S)
                m = valid.astype(np.float32)
                mask01[:, t, r, :] = m
                mask01[:, t, 2 + r, :] = m
            # fully-masked (pad) queries: open the self-key so den = exp(0)
            # = 1 exactly (pad x is all-zero -> score 0, V 0) -> no device-
            # side clamp needed before the reciprocal
            cnt = mask01[:, t, 0, :].sum(0) + mask01[:, t, 1, :].sum(0)
            for q in np.where(cnt == 0)[0]:
                r, ns = (0, q + 32) if q < 96 else (1, q - 96)
                mask01[ns, t, r, q] = 1.0
                mask01[ns, t, 2 + r, q] = 1.0

        per_core.append({
            "srcT": srcT.astype(bf16),
            "pos_emb": pos_emb,
            "mask01": mask01.reshape(128, NT, 512).astype(bf16),
        })

    # constants
    shared["ident"] = np.eye(128, dtype=np.float32)

    flags = {}
    for l in range(L):
        for nm in ("bq", "bk", "bv", "bfc", "b1", "b2"):
            flags[f"{nm}{l}"] = not np.allclose(shared[f"{nm}{l}"], 0.0)
        for nm in ("ln1", "ln2"):
            flags[f"{nm}{l}"] = not (
                np.allclose(shared[f"{nm}g{l}"], 1.0)
                and np.allclose(shared[f"{nm}b{l}"], 0.0)
            )
    return shared, per_core, flags


def assemble(core_outs):
    out = np.zeros((B, S, H), np.float32)
    for c in range(NCORES):
        b, q = divmod(c, NCORES // B)
        out[b, q * CHUNK:(q + 1) * CHUNK] = core_outs[c]
    return out


# ---------------------------------------------------------------------------
# Bass program
# ---------------------------------------------------------------------------

def _legalize_waits(nc):
    """This container's walrus codegen accepts only ONE sync-wait per compute
    instruction ("Too many sync wait commands"). Tile's scheduler emits
    multi-wait instructions, so split: keep the last wait on the instruction
    and carry earlier ones on same-engine NoOps inserted right before it."""
    import concourse.mybir as mybir

    for fn in nc.m.functions:
        for blk in fn.blocks:
            out = []
            changed = False
            for inst in blk.instructions:
                si = getattr(inst, "sync_info", None)
                waits = list(si.on_wait) if si is not None and si.on_wait else []
                if len(waits) > 1 and not isinstance(
                        inst, mybir.InstEventSemaphore):
                    for j, w in enumerate(waits[:-1]):
                        # NoOp lowers through the v3 codegen only; Activation
                        # and Pool go through v2 (no InstISA nop) -> use a
                        # 1-wait Drain there instead.
                        if inst.engine in (mybir.EngineType.Activation,
                                           mybir.EngineType.Pool):
                            nop = mybir.InstDrain(
                                name=f"{inst.name}-w{j}", ins=[], outs=[])
                        else:
                            nop = mybir.InstNoOp(
                                name=f"{inst.name}-w{j}", ins=[], outs=[])
                        nop.engine = inst.engine
                        nop.sync_info = mybir.SyncInfo(on_wait=[w], on_update=[])
                        out.append(nop)
                    inst.sync_info = mybir.SyncInfo(
                        on_wait=[waits[-1]], on_update=list(si.on_update or []))
                    changed = True
                out.append(inst)
            if changed:
                blk.instructions = out


def build_program(flags, legalize=True, stop_phase=99):
    import concourse.bass as bass
    import concourse.mybir as mybir
    import concourse.tile as tile

    f32 = mybir.dt.float32
    bf = mybir.dt.bfloat16
    AF = mybir.ActivationFunctionType
    ALU = mybir.AluOpType

    nc = bass.Bass()
    FT = H // 128          # 6 feature tiles
    KTE = FK // 128        # 4 embedding contraction tiles

    # ---- DRAM tensors ----
    D = {}
    names = []

    def din(name, shape, dt):
        D[name] = nc.dram_tensor(name, shape, dt, kind="ExternalInput")
        names.append(name)

    din("srcT", [FK, T_LOC], bf)
    din("pos_emb", [T_LOC, H], f32)
    din("mask01", [128, NT, 512], bf)
    din("ident", [128, 128], f32)
    din("wembT", [FK, H], bf)
    for l in range(L):
        for nm in ("wqT", "wkT", "wvT", "wfcT", "w1T", "w2T"):
            din(f"{nm}{l}", [H, H], bf)
        for nm in ("bq", "bk", "bv", "bfc", "b1", "b2"):
            if flags[f"{nm}{l}"]:
                din(f"{nm}{l}", [H], f32)
        for nm in ("ln1", "ln2"):
            if flags[f"{nm}{l}"]:
                din(f"{nm}g{l}", [H], f32)
                din(f"{nm}b{l}", [H], f32)
    out_d = nc.dram_tensor("out", [CHUNK, H], f32, kind="ExternalOutput")

    def bcast_ap(dram, n):
        return bass.AP(tensor=dram.tensor, offset=dram.offset, ap=[[0, 128], [1, n]])

    with tile.TileContext(nc) as tc:
        import contextlib
        with contextlib.ExitStack() as ctx:
            consts = ctx.enter_context(tc.tile_pool(name="consts", bufs=1))
            acts = ctx.enter_context(tc.tile_pool(name="acts", bufs=1))
            work = ctx.enter_context(tc.tile_pool(name="work", bufs=2))
            ppool = ctx.enter_context(tc.tile_pool(name="pp", bufs=4))
            rpool = ctx.enter_context(tc.tile_pool(name="rp", bufs=3))
            spool = ctx.enter_context(tc.tile_pool(name="sp", bufs=4))
            psg = ctx.enter_context(tc.tile_pool(name="psg", bufs=4 if STP_SHARED else 2, space="PSUM"))
            pbig = ctx.enter_context(tc.tile_pool(name="pbig", bufs=2, space="PSUM"))
            psc = ctx.enter_context(tc.tile_pool(name="psc", bufs=2, space="PSUM"))
            pst = (pbig if STP_SHARED else
                   ctx.enter_context(tc.tile_pool(name="pst", bufs=2, space="PSUM")))

            # ---- constants / inputs to SBUF ----
            ident_sb = consts.tile([128, 128], f32)
            nc.sync.dma_start(out=ident_sb, in_=D["ident"][:, :])
            ones64 = consts.tile([128, 64], bf)
            nc.vector.memset(ones64, 1.0)
            eps_sb = consts.tile([128, 1], f32)
            nc.vector.memset(eps_sb, 1e-5)
            # den-epsilon rank-1 pair: adds 1e-6 to every softmax denominator
            epsM = consts.tile([1, 64], bf)
            nc.vector.memset(epsM, 1e-6)
            ones_row = consts.tile([1, 128], bf)
            nc.vector.memset(ones_row, 1.0)

            mask_sb = consts.tile([128, NT, 512], bf)
            nc.sync.dma_start(out=mask_sb, in_=D["mask01"][:, :, :])

            srcT_sb = consts.tile([128, KTE, T_LOC], bf)
            for kt in range(KTE):
                nc.sync.dma_start(out=srcT_sb[:, kt, :],
                                  in_=D["srcT"][kt * 128:(kt + 1) * 128, :])
            pos_sb = consts.tile([128, NT, H], f32)
            for t in range(NT):
                nc.sync.dma_start(out=pos_sb[:, t, :],
                                  in_=D["pos_emb"][t * 128:(t + 1) * 128, :])

            # weights stream through a rotating pool: each is used once
            wpool = ctx.enter_context(tc.tile_pool(name="wpool", bufs=3))

            def load_w(name, kt=FT):
                wt = wpool.tile([128, kt, H], bf, name=f"{name}_sb", tag="wt")
                for k in range(kt):
                    nc.sync.dma_start(out=wt[:, k, :],
                                      in_=D[name][k * 128:(k + 1) * 128, :])
                return wt

            BIAS = {}
            for l in range(L):
                for nm in ("bq", "bk", "b1"):  # per-partition, feature-major
                    if flags[f"{nm}{l}"]:
                        BIAS[f"{nm}{l}"] = consts.tile([128, FT], f32, name=f"{nm}{l}_sb")
                        nc.sync.dma_start(
                            out=BIAS[f"{nm}{l}"],
                            in_=D[f"{nm}{l}"].rearrange("(kt p) -> p kt", p=128))
                for nm in ("bv", "bfc", "b2"):  # broadcast, token-major
                    if flags[f"{nm}{l}"]:
                        BIAS[f"{nm}{l}"] = consts.tile([128, H], f32, name=f"{nm}{l}_sb")
                        nc.sync.dma_start(
                            out=BIAS[f"{nm}{l}"], in_=bcast_ap(D[f"{nm}{l}"], H))
                for nm in ("ln1", "ln2"):
                    if flags[f"{nm}{l}"]:
                        for gb in ("g", "b"):
                            BIAS[f"{nm}{gb}{l}"] = consts.tile([128, H], f32, name=f"{nm}{gb}{l}_sb")
                            nc.sync.dma_start(
                                out=BIAS[f"{nm}{gb}{l}"],
                                in_=bcast_ap(D[f"{nm}{gb}{l}"], H))

            # ---- persistent activations ----
            x_tok = acts.tile([128, NT, H], f32)          # token-major f32
            qT = acts.tile([128, FT, T_LOC], bf)
            kTp = acts.tile([128, FT, KPAD], bf)
            V_sh = acts.tile([128, FT, H], bf)            # 6 shifted token tiles
            ctxT = acts.tile([128, FT, T_LOC], bf)
            H1T = acts.tile([128, FT, T_LOC], bf)

            # xT: feature-major bf16 with 32-col zero pad on each side (cols
            # [32, 672) hold tokens [0, 640)); a fresh generation per
            # transpose-set so the pool tracks lifetimes.
            veng = nc.gpsimd if USE_GPSIMD else nc.vector

            def new_xT(name):
                t_ = acts.tile([128, FT, H], bf, name=name, tag="xTslot")
                veng.memset(t_[:, :, 0:32], 0.0)
                veng.memset(t_[:, :, 32 + T_LOC:H], 0.0)
                return t_

            # ---- embedding ----
            wembT_sb = load_w("wembT", kt=KTE)
            for t in range(NT):
                for c0 in (0, 384):
                    ps = psg.tile([128, 384], f32, tag="gemm")
                    for kt in range(KTE):
                        nc.tensor.matmul(
                            ps, srcT_sb[:, kt, t * 128:(t + 1) * 128],
                            wembT_sb[:, kt, c0:c0 + 384],
                            start=(kt == 0), stop=(kt == KTE - 1))
                    nc.vector.tensor_add(
                        x_tok[:, t, c0:c0 + 384], ps, pos_sb[:, t, c0:c0 + 384])

            def transpose_set(dst, t):
                """PE-transpose x_tok tile t into dst[:, :, 128t:+128] (bf16).
                Drains alternate Scalar/Vector."""
                for g in range(2):
                    n_g = 4 if g == 0 else 2
                    trp = pbig.tile([128, 512], f32, tag="big")
                    for j in range(n_g):
                        ft = 4 * g + j
                        nc.tensor.transpose(
                            trp[:, j * 128:(j + 1) * 128],
                            x_tok[:, t, ft * 128:(ft + 1) * 128], ident_sb)
                    src = trp[:, 0:n_g * 128].rearrange("p (a b) -> p a b", b=128)
                    dsl = dst[:, 4 * g:4 * g + n_g, 32 + t * 128:32 + (t + 1) * 128]
                    if g == 0 and SCALAR_TR_DRAIN:
                        nc.scalar.copy(dsl, src)
                    else:
                        nc.vector.tensor_copy(dsl, src)

            xT = new_xT("x0T")
            for t in range(NT):
                transpose_set(xT, t)

            # ---- layers ----
            for l in range(L):
                base = 1 + 6 * l
                if stop_phase < base:
                    break
                # kTp pad memsets
                veng.memset(kTp[:, :, 0:32], 0.0)
                veng.memset(kTp[:, :, 32 + T_LOC:KPAD], 0.0)

                # q/k GEMMs (feature-major outputs); q drains on Scalar,
                # k drains on Vector
                wq = load_w(f"wqT{l}")
                wk = load_w(f"wkT{l}")
                for ft in range(FT):
                    for c0 in (0, 320):
                        psq = psg.tile([128, 384], f32, tag="gemm")
                        for kt in range(FT):
                            nc.tensor.matmul(
                                psq[:, 0:320], wq[:, kt, ft * 128:(ft + 1) * 128],
                                xT[:, kt, 32 + c0:32 + c0 + 320],
                                start=(kt == 0), stop=(kt == FT - 1))
                        if flags[f"bq{l}"]:
                            nc.vector.tensor_scalar_add(
                                qT[:, ft, c0:c0 + 320], psq[:, 0:320],
                                BIAS[f"bq{l}"][:, ft:ft + 1])
                        else:
                            nc.scalar.copy(qT[:, ft, c0:c0 + 320], psq[:, 0:320])
                        psk = psg.tile([128, 384], f32, tag="gemm")
                        for kt in range(FT):
                            nc.tensor.matmul(
                                psk[:, 0:320], wk[:, kt, ft * 128:(ft + 1) * 128],
                                xT[:, kt, 32 + c0:32 + c0 + 320],
                                start=(kt == 0), stop=(kt == FT - 1))
                        if flags[f"bk{l}"]:
                            nc.vector.tensor_scalar_add(
                                kTp[:, ft, 32 + c0:32 + c0 + 320], psk[:, 0:320],
                                BIAS[f"bk{l}"][:, ft:ft + 1])
                        else:
                            nc.vector.tensor_copy(
                                kTp[:, ft, 32 + c0:32 + c0 + 320], psk[:, 0:320])

                if stop_phase < base + 1:
                    break
                # V GEMM: shifted token windows (xT is padded, so window i is
                # simply padded cols [128i, 128i+128) -> uniform M=128)
                wv = load_w(f"wvT{l}")
                for i in range(FT):
                    for c0 in (0, 384):
                        psv = psg.tile([128, 384], f32, tag="gemm")
                        for kt in range(FT):
                            nc.tensor.matmul(
                                psv, xT[:, kt, 128 * i:128 * i + 128],
                                wv[:, kt, c0:c0 + 384],
                                start=(kt == 0), stop=(kt == FT - 1))
                        if flags[f"bv{l}"]:
                            nc.vector.tensor_add(
                                V_sh[:, i, c0:c0 + 384], psv,
                                BIAS[f"bv{l}"][:, c0:c0 + 384])
                        elif (i + (c0 > 0)) % 2 == 0:
                            nc.scalar.copy(V_sh[:, i, c0:c0 + 384], psv)
                        else:
                            nc.vector.tensor_copy(V_sh[:, i, c0:c0 + 384], psv)

                if stop_phase < base + 2:
                    break
                # attention: per (t, hp, hs) one stp bank holding [A|B']
                # scores (single stationary row-class per bank, like the
                # proven layout); exp+mask batched per hs, den/ctx with
                # K=128 contractions, reciprocal on DVE.
                for t in range(NT):
                    for hp in range(FT):
                        cps = psc.tile([128, 256], f32, tag="ctx")
                        for hs in range(2):
                            po = 64 * hs
                            h = 2 * hp + hs
                            stp = pst.tile([128, 256], f32,
                                           tag="big" if STP_SHARED else "st")
                            nc.tensor.matmul(
                                stp[:, 0:128],
                                kTp[po:po + 64, hp, 128 * t:128 * t + 128],
                                qT[po:po + 64, hp, 128 * t:128 * t + 128],
                                start=True, stop=True)
                            nc.tensor.matmul(
                                stp[:, 128:256],
                                kTp[po:po + 64, hp, 128 * t + 128:128 * t + 256],
                                qT[po:po + 64, hp, 128 * t:128 * t + 128],
                                start=True, stop=True)
                            if ATTN_DEPTH < 2:
                                continue
                            p = ppool.tile([128, 256], bf, tag="p")
                            nc.scalar.activation(p, stp, AF.Exp)
                            if ATTN_DEPTH < 3:
                                continue
                            veng.tensor_tensor(
                                out=p, in0=p, in1=mask_sb[:, t, 0:256],
                                op=ALU.mult)
                            if ATTN_DEPTH < 4:
                                continue
                            nc.tensor.matmul(
                                cps[po:po + 64, 128:256], ones64[0:128, :],
                                p[:, 0:128], start=True, stop=False)
                            nc.tensor.matmul(
                                cps[po:po + 64, 128:256], ones64[0:128, :],
                                p[:, 128:256], start=False, stop=not EPS_MATMUL)
                            if EPS_MATMUL:
                                nc.tensor.matmul(
                                    cps[po:po + 64, 128:256], epsM, ones_row,
                                    start=False, stop=True)
                            nc.tensor.matmul(
                                cps[po:po + 64, 0:128],
                                V_sh[0:128, t, 64 * h:64 * h + 64],
                                p[:, 0:128], start=True, stop=False)
                            nc.tensor.matmul(
                                cps[po:po + 64, 0:128],
                                V_sh[0:128, t + 1, 64 * h:64 * h + 64],
                                p[:, 128:256], start=False, stop=True)
                        if ATTN_DEPTH < 5:
                            continue
                        # den >= 1 everywhere (pad queries keep their self
                        # key open), so the clamp is numerically a no-op --
                        # but it paces the recip issue and measures ~5us
                        # faster than reading the den PSUM directly
                        nc.vector.tensor_scalar_max(
                            cps[:, 128:256], cps[:, 128:256], 1e-6)
                        if TT_DIVIDE:
                            # den >= 1e-6 via the eps matmul, so the divide
                            # never sees 0
                            nc.vector.tensor_tensor(
                                out=ctxT[:, hp, 128 * t:128 * t + 128],
                                in0=cps[:, 0:128], in1=cps[:, 128:256],
                                op=ALU.divide)
                        else:
                            rb = rpool.tile([128, 128], f32, tag="rb")
                            nc.vector.reciprocal(rb, cps[:, 128:256])
                            nc.vector.tensor_tensor(
                                out=ctxT[:, hp, 128 * t:128 * t + 128],
                                in0=cps[:, 0:128], in1=rb, op=ALU.mult)

                if stop_phase < base + 3:
                    break
                # fc + residual for all tiles first, then the LN1 batch:
                # keeps the Sqrt ACTs clustered after the attention Exps so
                # the LUT set switches only twice per layer.
                wfc = load_w(f"wfcT{l}")
                xT = new_xT(f"x1T{l}")
                for t in range(NT):
                    F = work.tile([128, H], f32, tag="F")
                    for c0 in (0, 384):
                        ps = psg.tile([128, 384], f32, tag="gemm")
                        for kt in range(FT):
                            nc.tensor.matmul(
                                ps, ctxT[:, kt, 128 * t:128 * t + 128],
                                wfc[:, kt, c0:c0 + 384],
                                start=(kt == 0), stop=(kt == FT - 1))
                        nc.vector.tensor_add(
                            F[:, c0:c0 + 384], ps, x_tok[:, t, c0:c0 + 384])
                        if flags[f"bfc{l}"]:
                            nc.vector.tensor_add(
                                F[:, c0:c0 + 384], F[:, c0:c0 + 384],
                                BIAS[f"bfc{l}"][:, c0:c0 + 384])
                    _layernorm(nc, tc, spool, F, x_tok, t, eps_sb,
                               BIAS.get(f"ln1g{l}"), BIAS.get(f"ln1b{l}"),
                               f32, AF, ALU)
                    transpose_set(xT, t)  # x1T reuses the xT slot

                if stop_phase < base + 4:
                    break
                # FFN: relu drains alternate Scalar (ACT Relu) / Vector (max 0)
                w1 = load_w(f"w1T{l}")
                for ft in range(FT):
                    for c0 in (0, 320):
                        ps = psg.tile([128, 384], f32, tag="gemm")
                        for kt in range(FT):
                            nc.tensor.matmul(
                                ps[:, 0:320], w1[:, kt, ft * 128:(ft + 1) * 128],
                                xT[:, kt, 32 + c0:32 + c0 + 320],
                                start=(kt == 0), stop=(kt == FT - 1))
                        if flags[f"b1{l}"]:
                            nc.scalar.activation(
                                H1T[:, ft, c0:c0 + 320], ps[:, 0:320], AF.Relu,
                                bias=BIAS[f"b1{l}"][:, ft:ft + 1])
                        elif (ft + (c0 > 0)) % 2 == 0:
                            nc.scalar.activation(
                                H1T[:, ft, c0:c0 + 320], ps[:, 0:320], AF.Relu)
                        else:
                            nc.vector.tensor_scalar_max(
                                H1T[:, ft, c0:c0 + 320], ps[:, 0:320], 0.0)
                if stop_phase < base + 5:
                    break
                w2 = load_w(f"w2T{l}")
                if l < L - 1:
                    xT = new_xT(f"x2T{l}")
                for t in range(NT):
                    F2 = work.tile([128, H], f32, tag="F")
                    for c0 in (0, 384):
                        ps = psg.tile([128, 384], f32, tag="gemm")
                        for kt in range(FT):
                            nc.tensor.matmul(
                                ps, H1T[:, kt, 128 * t:128 * t + 128],
                                w2[:, kt, c0:c0 + 384],
                                start=(kt == 0), stop=(kt == FT - 1))
                        nc.vector.tensor_add(
                            F2[:, c0:c0 + 384], ps, x_tok[:, t, c0:c0 + 384])
                        if flags[f"b2{l}"]:
                            nc.vector.tensor_add(
                                F2[:, c0:c0 + 384], F2[:, c0:c0 + 384],
                                BIAS[f"b2{l}"][:, c0:c0 + 384])
                    _layernorm(nc, tc, spool, F2, x_tok, t, eps_sb,
                               BIAS.get(f"ln2g{l}"), BIAS.get(f"ln2b{l}"),
                               f32, AF, ALU)
                    if l < L - 1:
                        transpose_set(xT, t)
                    else:
                        lo = max(128 * t, HALO) - 128 * t
                        hi = min(128 * t + 128, HALO + CHUNK) - 128 * t
                        nc.sync.dma_start(
                            out=out_d[128 * t + lo - HALO:128 * t + hi - HALO, :],
                            in_=x_tok[lo:hi, t, :])

            if stop_phase < 1 + 6 * L - 1:
                # truncated program: dump whatever x_tok holds
                for t in range(NT):
                    lo = max(128 * t, HALO) - 128 * t
                    hi = min(128 * t + 128, HALO + CHUNK) - 128 * t
                    nc.sync.dma_start(
                        out=out_d[128 * t + lo - HALO:128 * t + hi - HALO, :],
                        in_=x_tok[lo:hi, t, :])

    if legalize:
        _legalize_waits(nc)
    return nc, names


def _layernorm(nc, tc, spool, F, x_tok, t, eps_sb, g_bc, b_bc, f32, AF, ALU):
    stats = spool.tile([128, 3, 6], f32, tag="stats")
    for sg in range(3):
        nc.vector.bn_stats(stats[:, sg, :], F[:, sg * 256:(sg + 1) * 256])
    mv = spool.tile([128, 2], f32, tag="mv")
    nc.vector.bn_aggr(mv, stats)
    sd = spool.tile([128, 1], f32, tag="sd")
    nc.scalar.activation(sd, mv[:, 1:2], AF.Sqrt, bias=eps_sb[:, 0:1])
    rstd = spool.tile([128, 1], f32, tag="rstd")
    nc.vector.reciprocal(rstd, sd)
    nc.vector.tensor_scalar(
        out=x_tok[:, t, :], in0=F, scalar1=mv[:, 0:1], scalar2=rstd,
        op0=ALU.subtract, op1=ALU.mult)
    if g_bc is not None:
        nc.vector.tensor_tensor(
            out=x_tok[:, t, :], in0=x_tok[:, t, :], in1=g_bc, op=ALU.mult)
        nc.vector.tensor_tensor(
            out=x_tok[:, t, :], in0=x_tok[:, t, :], in1=b_bc, op=ALU.add)


def run_on_device(shared, per_core, flags, trace=False):
    from concourse.bass_utils import run_bass_kernel_spmd

    nc, names = build_program(flags)
    in_maps = []
    for c in range(NCORES):
        m = {}
        for n in names:
            src = per_core[c] if n in per_core[c] else shared
            m[n] = np.ascontiguousarray(src[n])
        in_maps.append(m)
    res = run_bass_kernel_spmd(nc, in_maps, core_ids=list(range(NCORES)),
                               trace=trace)
    return [r["out"] for r in res.results], res


def kernel(**inputs):
    shared, per_core, flags = host_prep(inputs)
    core_outs, _ = run_on_device(shared, per_core, flags)
    return assemble(core_outs)


# revision 64
# speedup vs baseline: 1.0182x; 1.0182x over previous
"""Trainium2 Bass kernel for a 2-layer Longformer-style encoder.

Model: B=2, S=2048, F=438, H=768, NH=12, HD=64, one-sided window w=32, L=2.

Sharding: 8 cores, data-parallel over (batch, sequence-quarter). Each core
computes 512 output tokens from a 640-token local window (64-token halo on
each side covers the 2-layer receptive field), so no collectives are needed.

Device algorithm per core (uniform SPMD, 640 local tokens):
  - x0 = srcT_pad.T @ W_embT + (pos_emb + b_emb)           [token-major f32]
  - per layer:
      xT   = transpose(x)  bf16                             [feature-major]
      qT   = W_qT'.T @ xT (+bq'), scaled by HD^-0.5 on host [feature-major]
      kTp  = W_kT.T @ xT (+bk), written at free-offset 32 into a
             768-wide padded buffer                         [feature-major]
      V_sh = shifted-window GEMM: tile i holds local tokens
             [128i-32, 128i+96)                             [token-major]
      per (query tile t, head pair hp, head half hs):
        2 score matmuls -> stp [128, 256] PSUM = [A | B'] where A covers
             keys [128t-32,128t+96) and B' a full 128 keys
             [128t+96,128t+224) so every stp element is a real score
             (one stationary row-class per PSUM bank - required: mixing
             row tile-positions in one bank hard-faults the device)
        p = exp(stp) in ONE [128,256] ACT (bf16 out), p *= mask01 (DVE)
        den += two K=128 ones-matmuls; ctx += V_sh[t|t+1].T @ p
        per hp: clamp den, rb = 1/den on DVE (not ACT: the Reciprocal
             LUT would thrash the Exp table set), ctxT = ctx * rb
      fc: F = ctxT.T @ W_fcT + residual; LN1 -> x1; transpose -> x1T bf16
      H1T = relu(W_1T.T @ x1T + b1)                         [feature-major]
      F2 = H1T.T @ W_2T (+b2) + x1; LN2 -> x2
  - out = x2[64:576]

Engine split: PE streams GEMMs; Scalar does exp, q-drains, half the relu
drains (Exp/Relu/Copy share one LUT set -> ~4 table loads total, vs 130
when ACT-Reciprocal was in the loop); Vector does k/v drains, masks, LN,
reciprocal. Measured on HW: 539.7us (predecessor) -> 312.8us.
"""

import numpy as np
import ml_dtypes

B, S, F_DIM, H, NH, HD, W_ONE, L = 2, 2048, 438, 768, 12, 64, 32, 2
NCORES = 8
CHUNK = 512          # output tokens per core
HALO = 64            # per side
T_LOC = CHUNK + 2 * HALO   # 640 local tokens
NT = T_LOC // 128          # 5 query tiles
KPAD = 768                 # padded key width (needs 128t+256 <= KPAD at t=4)
FK = 512                   # padded embedding contraction (438 -> 512)

bf16 = ml_dtypes.bfloat16

# Route mask-mult/memsets to the Pool (GpSimd) engine; flip off to bisect
# HW-only failures (CoreSim accepts both).
USE_GPSIMD = False
EPS_MATMUL = False     # den += 1e-6 via K=1 matmul (else: vector clamp)
SCALAR_TR_DRAIN = False  # 3-dim ACT Copy for transpose drains (else vector)
ATTN_DEPTH = 99
TT_DIVIDE = False       # single TT divide instead of recip+mult        # bisect: 1=scores 2=+exp 3=+mask 4=+den/ctx 5=+recip 6=all
STP_SHARED = False      # scores share the transpose PSUM pool (else own pool)
B_PRIME = False         # emit the B' (extended second-span) score matmuls


def _np(x):
    return np.asarray(x)


def host_prep(inputs):
    """Split full inputs into shared weight arrays + per-core arrays."""
    src_seq = _np(inputs["src_seq"]).astype(np.float32)
    src_pos = _np(inputs["src_pos"]).astype(np.int32)
    pos_table = _np(inputs["pos_table"]).astype(np.float32)

    shared = {}
    qscale = float(HD) ** -0.5

    W_emb = _np(inputs["W_emb"]).astype(np.float32)        # [H, F]
    WembT = np.zeros((FK, H), np.float32)
    WembT[:F_DIM] = W_emb.T
    shared["wembT"] = WembT.astype(bf16)

    for l in range(L):
        Wq = _np(inputs["Wq"])[l].astype(np.float32)
        Wk = _np(inputs["Wk"])[l].astype(np.float32)
        Wv = _np(inputs["Wv"])[l].astype(np.float32)
        Wfc = _np(inputs["Wfc"])[l].astype(np.float32)
        W1 = _np(inputs["W1"])[l].astype(np.float32)
        W2 = _np(inputs["W2"])[l].astype(np.float32)
        shared[f"wqT{l}"] = (Wq.T * qscale).astype(bf16)   # [H_in, H_out]
        shared[f"wkT{l}"] = Wk.T.astype(bf16)
        shared[f"wvT{l}"] = Wv.T.astype(bf16)
        shared[f"wfcT{l}"] = Wfc.T.astype(bf16)
        shared[f"w1T{l}"] = W1.T.astype(bf16)
        shared[f"w2T{l}"] = W2.T.astype(bf16)
        shared[f"bq{l}"] = (_np(inputs["bq"])[l].astype(np.float32) * qscale)
        shared[f"bk{l}"] = _np(inputs["bk"])[l].astype(np.float32)
        shared[f"bv{l}"] = _np(inputs["bv"])[l].astype(np.float32)
        shared[f"bfc{l}"] = _np(inputs["bfc"])[l].astype(np.float32)
        shared[f"b1{l}"] = _np(inputs["b1"])[l].astype(np.float32)
        shared[f"b2{l}"] = _np(inputs["b2"])[l].astype(np.float32)
        shared[f"ln1g{l}"] = _np(inputs["ln1_g"])[l].astype(np.float32)
        shared[f"ln1b{l}"] = _np(inputs["ln1_b"])[l].astype(np.float32)
        shared[f"ln2g{l}"] = _np(inputs["ln2_g"])[l].astype(np.float32)
        shared[f"ln2b{l}"] = _np(inputs["ln2_b"])[l].astype(np.float32)

    b_emb = _np(inputs["b_emb"]).astype(np.float32)

    per_core = []
    for c in range(NCORES):
        b, q = divmod(c, NCORES // B)
        gstart = q * CHUNK - HALO
        lo, hi = max(gstart, 0), min(gstart + T_LOC, S)

        src_halo = np.zeros((T_LOC, F_DIM), np.float32)
        src_halo[lo - gstart: hi - gstart] = src_seq[b, lo:hi]
        srcT = np.zeros((FK, T_LOC), np.float32)
        srcT[:F_DIM] = src_halo.T

        pos_emb = np.zeros((T_LOC, H), np.float32)
        pos_emb[lo - gstart: hi - gstart] = pos_table[src_pos[b, lo:hi]]
        pos_emb += b_emb[None, :]

        # multiplicative 0/1 mask, n-major, [n, region(A|B'), q] per tile,
        # duplicated for the two heads of a feature tile: [128, NT, 512]
        # A : key_local = 128t - 32 + n  (n in [0,128))
        # B': key_local = 128t + 96 + n  (n in [0,128))
        mask01 = np.zeros((128, NT, 4, 128), np.float32)
        n = np.arange(128)[:, None]
        qq = np.arange(128)[None, :]
        for t in range(NT):
            for r, kbase in ((0, 128 * t - 32), (1, 128 * t + 96)):
                kl = kbase + n
                kg = gstart + kl
                band = np.abs(kl - (128 * t + qq)) <= W_ONE
                valid = band & (kl >= 0) & (kl < T_LOC) & (kg >= 0) & (kg < S)
                m = valid.astype(np.float32)
                mask01[:, t, r, :] = m
                mask01[:, t, 2 + r, :] = m
            # fully-masked (pad) queries: open the self-key so den = exp(0)
            # = 1 exactly (pad x is all-zero -> score 0, V 0) -> no device-
            # side clamp needed before the reciprocal
            cnt = mask01[:, t, 0, :].sum(0) + mask01[:, t, 1, :].sum(0)
            for q in np.where(cnt == 0)[0]:
                r, ns = (0, q + 32) if q < 96 else (1, q - 96)
                mask01[ns, t, r, q] = 1.0
                mask01[ns, t, 2 + r, q] = 1.0

        per_core.append({
            "srcT": srcT.astype(bf16),
            "pos_emb": pos_emb,
            "mask01": mask01.reshape(128, NT, 512).astype(bf16),
        })

    # constants
    shared["ident"] = np.eye(128, dtype=np.float32)

    flags = {}
    for l in range(L):
        for nm in ("bq", "bk", "bv", "bfc", "b1", "b2"):
            flags[f"{nm}{l}"] = not np.allclose(shared[f"{nm}{l}"], 0.0)
        for nm in ("ln1", "ln2"):
            flags[f"{nm}{l}"] = not (
                np.allclose(shared[f"{nm}g{l}"], 1.0)
                and np.allclose(shared[f"{nm}b{l}"], 0.0)
            )
    return shared, per_core, flags


def assemble(core_outs):
    out = np.zeros((B, S, H), np.float32)
    for c in range(NCORES):
        b, q = divmod(c, NCORES // B)
        out[b, q * CHUNK:(q + 1) * CHUNK] = core_outs[c]
    return out


# ---------------------------------------------------------------------------
# Bass program
# ---------------------------------------------------------------------------

def _legalize_waits(nc):
    """This container's walrus codegen accepts only ONE sync-wait per compute
    instruction ("Too many sync wait commands"). Tile's scheduler emits
    multi-wait instructions, so split: keep the last wait on the instruction
    and carry earlier ones on same-engine NoOps inserted right before it."""
    import concourse.mybir as mybir

    for fn in nc.m.functions:
        for blk in fn.blocks:
            out = []
            changed = False
            for inst in blk.instructions:
                si = getattr(inst, "sync_info", None)
                waits = list(si.on_wait) if si is not None and si.on_wait else []
                if len(waits) > 1 and not isinstance(
                        inst, mybir.InstEventSemaphore):
                    for j, w in enumerate(waits[:-1]):
                        # NoOp lowers through the v3 codegen only; Activation
                        # and Pool go through v2 (no InstISA nop) -> use a
                        # 1-wait Drain there instead.
                        if inst.engine in (mybir.EngineType.Activation,
                                           mybir.EngineType.Pool):
                            nop = mybir.InstDrain(
                                name=f"{inst.name}-w{j}", ins=[], outs=[])
                        else:
                            nop = mybir.InstNoOp(
                                name=f"{inst.name}-w{j}", ins=[], outs=[])
                        nop.engine = inst.engine
                        nop.sync_info = mybir.SyncInfo(on_wait=[w], on_update=[])
                        out.append(nop)
                    inst.sync_info = mybir.SyncInfo(
                        on_wait=[waits[-1]], on_update=list(si.on_update or []))
                    changed = True
                out.append(inst)
            if changed:
                blk.instructions = out


def build_program(flags, legalize=True, stop_phase=99):
    import concourse.bass as bass
    import concourse.mybir as mybir
    import concourse.tile as tile

    f32 = mybir.dt.float32
    bf = mybir.dt.bfloat16
    AF = mybir.ActivationFunctionType
    ALU = mybir.AluOpType

    nc = bass.Bass()
    FT = H // 128          # 6 feature tiles
    KTE = FK // 128        # 4 embedding contraction tiles

    # ---- DRAM tensors ----
    D = {}
    names = []

    def din(name, shape, dt):
        D[name] = nc.dram_tensor(name, shape, dt, kind="ExternalInput")
        names.append(name)

    din("srcT", [FK, T_LOC], bf)
    din("pos_emb", [T_LOC, H], f32)
    din("mask01", [128, NT, 512], bf)
    din("ident", [128, 128], f32)
    din("wembT", [FK, H], bf)
    for l in range(L):
        for nm in ("wqT", "wkT", "wvT", "wfcT", "w1T", "w2T"):
            din(f"{nm}{l}", [H, H], bf)
        for nm in ("bq", "bk", "bv", "bfc", "b1", "b2"):
            if flags[f"{nm}{l}"]:
                din(f"{nm}{l}", [H], f32)
        for nm in ("ln1", "ln2"):
            if flags[f"{nm}{l}"]:
                din(f"{nm}g{l}", [H], f32)
                din(f"{nm}b{l}", [H], f32)
    out_d = nc.dram_tensor("out", [CHUNK, H], f32, kind="ExternalOutput")

    def bcast_ap(dram, n):
        return bass.AP(tensor=dram.tensor, offset=dram.offset, ap=[[0, 128], [1, n]])

    with tile.TileContext(nc) as tc:
        import contextlib
        with contextlib.ExitStack() as ctx:
            consts = ctx.enter_context(tc.tile_pool(name="consts", bufs=1))
            acts = ctx.enter_context(tc.tile_pool(name="acts", bufs=1))
            work = ctx.enter_context(tc.tile_pool(name="work", bufs=2))
            ppool = ctx.enter_context(tc.tile_pool(name="pp", bufs=4))
            rpool = ctx.enter_context(tc.tile_pool(name="rp", bufs=3))
            spool = ctx.enter_context(tc.tile_pool(name="sp", bufs=4))
            psg = ctx.enter_context(tc.tile_pool(name="psg", bufs=4 if STP_SHARED else 2, space="PSUM"))
            pbig = ctx.enter_context(tc.tile_pool(name="pbig", bufs=2, space="PSUM"))
            psc = ctx.enter_context(tc.tile_pool(name="psc", bufs=2, space="PSUM"))
            pst = (pbig if STP_SHARED else
                   ctx.enter_context(tc.tile_pool(name="pst", bufs=2, space="PSUM")))

            # ---- constants / inputs to SBUF ----
            ident_sb = consts.tile([128, 128], f32)
            nc.sync.dma_start(out=ident_sb, in_=D["ident"][:, :])
            ones64 = consts.tile([128, 64], bf)
            nc.vector.memset(ones64, 1.0)
            eps_sb = consts.tile([128, 1], f32)
            nc.vector.memset(eps_sb, 1e-5)
            # den-epsilon rank-1 pair: adds 1e-6 to every softmax denominator
            epsM = consts.tile([1, 64], bf)
            nc.vector.memset(epsM, 1e-6)
            ones_row = consts.tile([1, 128], bf)
            nc.vector.memset(ones_row, 1.0)

            mask_sb = consts.tile([128, NT, 512], bf)
            nc.sync.dma_start(out=mask_sb, in_=D["mask01"][:, :, :])

            srcT_sb = consts.tile([128, KTE, T_LOC], bf)
            for kt in range(KTE):
                nc.sync.dma_start(out=srcT_sb[:, kt, :],
                                  in_=D["srcT"][kt * 128:(kt + 1) * 128, :])
            pos_sb = consts.tile([128, NT, H], f32)
            for t in range(NT):
                nc.sync.dma_start(out=pos_sb[:, t, :],
                                  in_=D["pos_emb"][t * 128:(t + 1) * 128, :])

            # weights stream through a rotating pool: each is used once
            wpool = ctx.enter_context(tc.tile_pool(name="wpool", bufs=4))

            def load_w(name, kt=FT):
                wt = wpool.tile([128, kt, H], bf, name=f"{name}_sb", tag="wt")
                for k in range(kt):
                    nc.sync.dma_start(out=wt[:, k, :],
                                      in_=D[name][k * 128:(k + 1) * 128, :])
                return wt

            BIAS = {}
            for l in range(L):
                for nm in ("bq", "bk", "b1"):  # per-partition, feature-major
                    if flags[f"{nm}{l}"]:
                        BIAS[f"{nm}{l}"] = consts.tile([128, FT], f32, name=f"{nm}{l}_sb")
                        nc.sync.dma_start(
                            out=BIAS[f"{nm}{l}"],
                            in_=D[f"{nm}{l}"].rearrange("(kt p) -> p kt", p=128))
                for nm in ("bv", "bfc", "b2"):  # broadcast, token-major
                    if flags[f"{nm}{l}"]:
                        BIAS[f"{nm}{l}"] = consts.tile([128, H], f32, name=f"{nm}{l}_sb")
                        nc.sync.dma_start(
                            out=BIAS[f"{nm}{l}"], in_=bcast_ap(D[f"{nm}{l}"], H))
                for nm in ("ln1", "ln2"):
                    if flags[f"{nm}{l}"]:
                        for gb in ("g", "b"):
                            BIAS[f"{nm}{gb}{l}"] = consts.tile([128, H], f32, name=f"{nm}{gb}{l}_sb")
                            nc.sync.dma_start(
                                out=BIAS[f"{nm}{gb}{l}"],
                                in_=bcast_ap(D[f"{nm}{gb}{l}"], H))

            # ---- persistent activations ----
            x_tok = acts.tile([128, NT, H], f32)          # token-major f32
            qT = acts.tile([128, FT, T_LOC], bf)
            kTp = acts.tile([128, FT, KPAD], bf)
            V_sh = acts.tile([128, FT, H], bf)            # 6 shifted token tiles
            ctxT = acts.tile([128, FT, T_LOC], bf)
            H1T = acts.tile([128, FT, T_LOC], bf)

            # xT: feature-major bf16 with 32-col zero pad on each side (cols
            # [32, 672) hold tokens [0, 640)); a fresh generation per
            # transpose-set so the pool tracks lifetimes.
            veng = nc.gpsimd if USE_GPSIMD else nc.vector

            def new_xT(name):
                t_ = acts.tile([128, FT, H], bf, name=name, tag="xTslot")
                veng.memset(t_[:, :, 0:32], 0.0)
                veng.memset(t_[:, :, 32 + T_LOC:H], 0.0)
                return t_

            # ---- embedding ----
            wembT_sb = load_w("wembT", kt=KTE)
            for t in range(NT):
                for c0 in (0, 384):
                    ps = psg.tile([128, 384], f32, tag="gemm")
                    for kt in range(KTE):
                        nc.tensor.matmul(
                            ps, srcT_sb[:, kt, t * 128:(t + 1) * 128],
                            wembT_sb[:, kt, c0:c0 + 384],
                            start=(kt == 0), stop=(kt == KTE - 1))
                    nc.vector.tensor_add(
                        x_tok[:, t, c0:c0 + 384], ps, pos_sb[:, t, c0:c0 + 384])

            def transpose_set(dst, t):
                """PE-transpose x_tok tile t into dst[:, :, 128t:+128] (bf16).
                Drains alternate Scalar/Vector."""
                for g in range(2):
                    n_g = 4 if g == 0 else 2
                    trp = pbig.tile([128, 512], f32, tag="big")
                    for j in range(n_g):
                        ft = 4 * g + j
                        nc.tensor.transpose(
                            trp[:, j * 128:(j + 1) * 128],
                            x_tok[:, t, ft * 128:(ft + 1) * 128], ident_sb)
                    src = trp[:, 0:n_g * 128].rearrange("p (a b) -> p a b", b=128)
                    dsl = dst[:, 4 * g:4 * g + n_g, 32 + t * 128:32 + (t + 1) * 128]
                    if g == 0 and SCALAR_TR_DRAIN:
                        nc.scalar.copy(dsl, src)
                    else:
                        nc.vector.tensor_copy(dsl, src)

            xT = new_xT("x0T")
            for t in range(NT):
                transpose_set(xT, t)

            # ---- layers ----
            for l in range(L):
                base = 1 + 6 * l
                if stop_phase < base:
                    break
                # kTp pad memsets
                veng.memset(kTp[:, :, 0:32], 0.0)
                veng.memset(kTp[:, :, 32 + T_LOC:KPAD], 0.0)

                # q/k GEMMs (feature-major outputs); q drains on Scalar,
                # k drains on Vector
                wq = load_w(f"wqT{l}")
                wk = load_w(f"wkT{l}")
                for ft in range(FT):
                    for c0 in (0, 320):
                        psq = psg.tile([128, 384], f32, tag="gemm")
                        for kt in range(FT):
                            nc.tensor.matmul(
                                psq[:, 0:320], wq[:, kt, ft * 128:(ft + 1) * 128],
                                xT[:, kt, 32 + c0:32 + c0 + 320],
                                start=(kt == 0), stop=(kt == FT - 1))
                        if flags[f"bq{l}"]:
                            nc.vector.tensor_scalar_add(
                                qT[:, ft, c0:c0 + 320], psq[:, 0:320],
                                BIAS[f"bq{l}"][:, ft:ft + 1])
                        else:
                            nc.scalar.copy(qT[:, ft, c0:c0 + 320], psq[:, 0:320])
                        psk = psg.tile([128, 384], f32, tag="gemm")
                        for kt in range(FT):
                            nc.tensor.matmul(
                                psk[:, 0:320], wk[:, kt, ft * 128:(ft + 1) * 128],
                                xT[:, kt, 32 + c0:32 + c0 + 320],
                                start=(kt == 0), stop=(kt == FT - 1))
                        if flags[f"bk{l}"]:
                            nc.vector.tensor_scalar_add(
                                kTp[:, ft, 32 + c0:32 + c0 + 320], psk[:, 0:320],
                                BIAS[f"bk{l}"][:, ft:ft + 1])
                        else:
                            nc.vector.tensor_copy(
                                kTp[:, ft, 32 + c0:32 + c0 + 320], psk[:, 0:320])

                if stop_phase < base + 1:
                    break
                # V GEMM: shifted token windows (xT is padded, so window i is
                # simply padded cols [128i, 128i+128) -> uniform M=128)
                wv = load_w(f"wvT{l}")
                for i in range(FT):
                    for c0 in (0, 384):
                        psv = psg.tile([128, 384], f32, tag="gemm")
                        for kt in range(FT):
                            nc.tensor.matmul(
                                psv, xT[:, kt, 128 * i:128 * i + 128],
                                wv[:, kt, c0:c0 + 384],
                                start=(kt == 0), stop=(kt == FT - 1))
                        if flags[f"bv{l}"]:
                            nc.vector.tensor_add(
                                V_sh[:, i, c0:c0 + 384], psv,
                                BIAS[f"bv{l}"][:, c0:c0 + 384])
                        elif (i + (c0 > 0)) % 2 == 0:
                            nc.scalar.copy(V_sh[:, i, c0:c0 + 384], psv)
                        else:
                            nc.vector.tensor_copy(V_sh[:, i, c0:c0 + 384], psv)

                if stop_phase < base + 2:
                    break
                # attention: per (t, hp, hs) one stp bank holding [A|B']
                # scores (single stationary row-class per bank, like the
                # proven layout); exp+mask batched per hs, den/ctx with
                # K=128 contractions, reciprocal on DVE.
                for t in range(NT):
                    for hp in range(FT):
                        cps = psc.tile([128, 256], f32, tag="ctx")
                        for hs in range(2):
                            po = 64 * hs
                            h = 2 * hp + hs
                            stp = pst.tile([128, 256], f32,
                                           tag="big" if STP_SHARED else "st")
                            nc.tensor.matmul(
                                stp[:, 0:128],
                                kTp[po:po + 64, hp, 128 * t:128 * t + 128],
                                qT[po:po + 64, hp, 128 * t:128 * t + 128],
                                start=True, stop=True)
                            nc.tensor.matmul(
                                stp[:, 128:256],
                                kTp[po:po + 64, hp, 128 * t + 128:128 * t + 256],
                                qT[po:po + 64, hp, 128 * t:128 * t + 128],
                                start=True, stop=True)
                            if ATTN_DEPTH < 2:
                                continue
                            p = ppool.tile([128, 256], bf, tag="p")
                            nc.scalar.activation(p, stp, AF.Exp)
                            if ATTN_DEPTH < 3:
                                continue
                            veng.tensor_tensor(
                                out=p, in0=p, in1=mask_sb[:, t, 0:256],
                                op=ALU.mult)
                            if ATTN_DEPTH < 4:
                                continue
                            nc.tensor.matmul(
                                cps[po:po + 64, 128:256], ones64[0:128, :],
                                p[:, 0:128], start=True, stop=False)
                            nc.tensor.matmul(
                                cps[po:po + 64, 128:256], ones64[0:128, :],
                                p[:, 128:256], start=False, stop=not EPS_MATMUL)
                            if EPS_MATMUL:
                                nc.tensor.matmul(
                                    cps[po:po + 64, 128:256], epsM, ones_row,
                                    start=False, stop=True)
                            nc.tensor.matmul(
                                cps[po:po + 64, 0:128],
                                V_sh[0:128, t, 64 * h:64 * h + 64],
                                p[:, 0:128], start=True, stop=False)
                            nc.tensor.matmul(
                                cps[po:po + 64, 0:128],
                                V_sh[0:128, t + 1, 64 * h:64 * h + 64],
                                p[:, 128:256], start=False, stop=True)
                        if ATTN_DEPTH < 5:
                            continue
                        # den >= 1 everywhere (pad queries keep their self
                        # key open), so the clamp is numerically a no-op --
                        # but it paces the recip issue and measures ~5us
                        # faster than reading the den PSUM directly
                        nc.vector.tensor_scalar_max(
                            cps[:, 128:256], cps[:, 128:256], 1e-6)
                        if TT_DIVIDE:
                            # den >= 1e-6 via the eps matmul, so the divide
                            # never sees 0
                            nc.vector.tensor_tensor(
                                out=ctxT[:, hp, 128 * t:128 * t + 128],
                                in0=cps[:, 0:128], in1=cps[:, 128:256],
                                op=ALU.divide)
                        else:
                            rb = rpool.tile([128, 128], f32, tag="rb")
                            nc.vector.reciprocal(rb, cps[:, 128:256])
                            nc.vector.tensor_tensor(
                                out=ctxT[:, hp, 128 * t:128 * t + 128],
                                in0=cps[:, 0:128], in1=rb, op=ALU.mult)

                if stop_phase < base + 3:
                    break
                # fc + residual for all tiles first, then the LN1 batch:
                # keeps the Sqrt ACTs clustered after the attention Exps so
                # the LUT set switches only twice per layer.
                wfc = load_w(f"wfcT{l}")
                xT = new_xT(f"x1T{l}")
                for t in range(NT):
                    F = work.tile([128, H], f32, tag="F")
                    for c0 in (0, 384):
                        ps = psg.tile([128, 384], f32, tag="gemm")
                        for kt in range(FT):
                            nc.tensor.matmul(
                                ps, ctxT[:, kt, 128 * t:128 * t + 128],
                                wfc[:, kt, c0:c0 + 384],
                                start=(kt == 0), stop=(kt == FT - 1))
                        nc.vector.tensor_add(
                            F[:, c0:c0 + 384], ps, x_tok[:, t, c0:c0 + 384])
                        if flags[f"bfc{l}"]:
                            nc.vector.tensor_add(
                                F[:, c0:c0 + 384], F[:, c0:c0 + 384],
                                BIAS[f"bfc{l}"][:, c0:c0 + 384])
                    _layernorm(nc, tc, spool, F, x_tok, t, eps_sb,
                               BIAS.get(f"ln1g{l}"), BIAS.get(f"ln1b{l}"),
                               f32, AF, ALU)
                    transpose_set(xT, t)  # x1T reuses the xT slot

                if stop_phase < base + 4:
                    break
                # FFN: relu drains alternate Scalar (ACT Relu) / Vector (max 0)
                w1 = load_w(f"w1T{l}")
                for ft in range(FT):
                    for c0 in (0, 320):
                        ps = psg.tile([128, 384], f32, tag="gemm")
                        for kt in range(FT):
                            nc.tensor.matmul(
                                ps[:, 0:320], w1[:, kt, ft * 128:(ft + 1) * 128],
                                xT[:, kt, 32 + c0:32 + c0 + 320],
                                start=(kt == 0), stop=(kt == FT - 1))
                        if flags[f"b1{l}"]:
                            nc.scalar.activation(
                                H1T[:, ft, c0:c0 + 320], ps[:, 0:320], AF.Relu,
                                bias=BIAS[f"b1{l}"][:, ft:ft + 1])
                        elif (ft + (c0 > 0)) % 2 == 0:
                            nc.scalar.activation(
                                H1T[:, ft, c0:c0 + 320], ps[:, 0:320], AF.Relu)
                        else:
                            nc.vector.tensor_scalar_max(
                                H1T[:, ft, c0:c0 + 320], ps[:, 0:320], 0.0)
                if stop_phase < base + 5:
                    break
                w2 = load_w(f"w2T{l}")
                if l < L - 1:
                    xT = new_xT(f"x2T{l}")
                for t in range(NT):
                    F2 = work.tile([128, H], f32, tag="F")
                    for c0 in (0, 384):
                        ps = psg.tile([128, 384], f32, tag="gemm")
                        for kt in range(FT):
                            nc.tensor.matmul(
                                ps, H1T[:, kt, 128 * t:128 * t + 128],
                                w2[:, kt, c0:c0 + 384],
                                start=(kt == 0), stop=(kt == FT - 1))
                        nc.vector.tensor_add(
                            F2[:, c0:c0 + 384], ps, x_tok[:, t, c0:c0 + 384])
                        if flags[f"b2{l}"]:
                            nc.vector.tensor_add(
                                F2[:, c0:c0 + 384], F2[:, c0:c0 + 384],
                                BIAS[f"b2{l}"][:, c0:c0 + 384])
                    _layernorm(nc, tc, spool, F2, x_tok, t, eps_sb,
                               BIAS.get(f"ln2g{l}"), BIAS.get(f"ln2b{l}"),
                               f32, AF, ALU)
                    if l < L - 1:
                        transpose_set(xT, t)
                    else:
                        lo = max(128 * t, HALO) - 128 * t
                        hi = min(128 * t + 128, HALO + CHUNK) - 128 * t
                        nc.sync.dma_start(
                            out=out_d[128 * t + lo - HALO:128 * t + hi - HALO, :],
                            in_=x_tok[lo:hi, t, :])

            if stop_phase < 1 + 6 * L - 1:
                # truncated program: dump whatever x_tok holds
                for t in range(NT):
                    lo = max(128 * t, HALO) - 128 * t
                    hi = min(128 * t + 128, HALO + CHUNK) - 128 * t
                    nc.sync.dma_start(
                        out=out_d[128 * t + lo - HALO:128 * t + hi - HALO, :],
                        in_=x_tok[lo:hi, t, :])

    if legalize:
        _legalize_waits(nc)
    return nc, names


def _layernorm(nc, tc, spool, F, x_tok, t, eps_sb, g_bc, b_bc, f32, AF, ALU):
    stats = spool.tile([128, 3, 6], f32, tag="stats")
    for sg in range(3):
        nc.vector.bn_stats(stats[:, sg, :], F[:, sg * 256:(sg + 1) * 256])
    mv = spool.tile([128, 2], f32, tag="mv")
    nc.vector.bn_aggr(mv, stats)
    sd = spool.tile([128, 1], f32, tag="sd")
    nc.scalar.activation(sd, mv[:, 1:2], AF.Sqrt, bias=eps_sb[:, 0:1])
    rstd = spool.tile([128, 1], f32, tag="rstd")
    nc.vector.reciprocal(rstd, sd)
    nc.vector.tensor_scalar(
        out=x_tok[:, t, :], in0=F, scalar1=mv[:, 0:1], scalar2=rstd,
        op0=ALU.subtract, op1=ALU.mult)
    if g_bc is not None:
        nc.vector.tensor_tensor(
            out=x_tok[:, t, :], in0=x_tok[:, t, :], in1=g_bc, op=ALU.mult)
        nc.vector.tensor_tensor(
            out=x_tok[:, t, :], in0=x_tok[:, t, :], in1=b_bc, op=ALU.add)


def run_on_device(shared, per_core, flags, trace=False):
    from concourse.bass_utils import run_bass_kernel_spmd

    nc, names = build_program(flags)
    in_maps = []
    for c in range(NCORES):
        m = {}
        for n in names:
            src = per_core[c] if n in per_core[c] else shared
            m[n] = np.ascontiguousarray(src[n])
        in_maps.append(m)
    res = run_bass_kernel_spmd(nc, in_maps, core_ids=list(range(NCORES)),
                               trace=trace)
    return [r["out"] for r in res.results], res


def kernel(**inputs):
    shared, per_core, flags = host_prep(inputs)
    core_outs, _ = run_on_device(shared, per_core, flags)
    return assemble(core_outs)
